# revision 2
# baseline (speedup 1.0000x reference)
"""SATD-style custom loss on 8 Trainium2 NeuronCores (v2).

Computes sum(|H8 @ (original - pred)|) where H8 is the 8x8 Sylvester
Hadamard matrix applied along dim -2 of [B, C, 8, 8] blocks.

Strategy (v2): pure data parallel over the block-batch dim (8 shards).
  - Host computes the residual d = original - pred and casts it to
    fp8e4 (e4m3): 8x less HBM traffic than shipping both fp32 tensors
    (the residual is the only thing the loss depends on; quantization
    costs ~3e-4 rel err on the loss, gate is 2e-2).
  - Layout per core: [128, 98304] fp8, partition k = g*8+j (16
    block-groups x 8 Hadamard input rows), col n = b*8+w.
  - DMA: SP HWDGE + gpsimd SWDGE alternate 512KB slab transfers.
  - PE: 64x64 array packing (4 independent PE tiles). The H8 transform
    is block-diagonal with K=8, so a 64x64 tile covers 8 block-groups.
    Tile (s,d) reads SBUF partitions 64s:64s+64 and writes PSUM
    partitions 64d:64d+64; per 512-col PSUM bank, 2 tile-matmuls
    (one per PSUM half) fill the bank from the two SBUF halves.
    4 tiles run concurrently -> 2x PE column throughput, so the PE
    does not starve the drain engines even when HAM-throttled.
    (32x32 with 16 tiles would be 4x, but SBUF base partition 96 -
    PE quadrant 3 - is broken on TRN2, so 32-partition stripe s=3
    cannot feed the PE.)
  - PSUM drain: [128, 1024] fp32 chunks (two adjacent banks, a single
    contiguous PSUM read) abs-summed by ACT (activation(Abs, accum_out))
    and DVE (tensor_reduce add with apply_absolute_value), greedily
    balanced by measured per-chunk cost. TRN2 allows only one PSUM
    input per instruction and gpsimd cannot touch PSUM, so these two
    engines are the drain ceiling.
  - Final [128,1] partials DMA'd out; host sums 8x128 in float64.
"""

import numpy as np
import ml_dtypes

import concourse.bacc as bacc
import concourse.bass as bass
import concourse.mybir as mybir
from concourse.bass_utils import run_bass_kernel_spmd
from concourse.tile import TileContext

# Problem shape (hardcoded; kernel.py must be self-contained).
N_BLOCKS = 524288
C = 3
N_CORES = 8
BLOCKS_PER_CORE = (N_BLOCKS // N_CORES) * C  # 196608 8x8 blocks
G = 16  # block-groups stacked on partitions (16 * 8 = 128)
BPG = BLOCKS_PER_CORE // G  # 12288 blocks per group
P = 128
NCOLS = BPG * 8  # 98304 moving columns (b*8+w)
BANK_N = 512  # fp32 cols per PSUM bank
TILE_N = 1024  # drain granularity: two adjacent banks
TILES = NCOLS // TILE_N  # 96 double-bank tiles
SLAB_N = 4096  # cols per full DMA slab (512 KiB)

ACT_NS = 1278.0  # measured: activation + accumulator-read per [128,1024]
DVE_NS = 1181.0  # measured tensor_reduce on [128,1024] from PSUM

F8 = ml_dtypes.float8_e4m3  # matches mybir dt.float8e4


def _hadamard8() -> np.ndarray:
    H = np.array([[1.0]], dtype=np.float32)
    while H.shape[0] < 8:
        H = np.block([[H, H], [H, -H]])
    return H


def _build_program() -> bass.Bass:
    nc = bacc.Bacc("TRN2", debug=False, num_devices=N_CORES)
    dt = mybir.dt

    x_dram = nc.declare_dram_parameter("x", [P, NCOLS], dt.float8e4, isOutput=False)
    w_dram = nc.declare_dram_parameter("w", [P, 64], dt.float8e4, isOutput=False)
    out_dram = nc.declare_dram_parameter("out", [P, 1], dt.float32, isOutput=True)

    with TileContext(nc) as tc:
        with (
            tc.tile_pool(name="io", bufs=8) as io_pool,
            tc.tile_pool(name="wpool", bufs=1) as w_pool,
            tc.tile_pool(name="dump", bufs=4) as dump_pool,
            tc.tile_pool(name="acc", bufs=1) as acc_pool,
            tc.tile_pool(name="psum", bufs=4, space="PSUM") as psum_pool,
        ):
            wt = w_pool.tile([P, 64], dt.float8e4)
            nc.sync.dma_start(out=wt[:], in_=w_dram[:, :])

            acc = acc_pool.tile([P, TILES], dt.float32)

            # slab plan: first slabs arrive in small pieces so the first
            # matmuls + drains start as soon as possible; rest are 4096-col
            # (512 KiB) transfers alternating between the two DMA paths
            # (SP HWDGE, gpsimd SWDGE). ACT/DVE never trigger DMA - they
            # are drain-bound.
            # All x DMA on HWDGE rings: sync carries the stream; scalar
            # (idle until its first drain chunk ~4us in) carries the first
            # two pieces so the pipeline primes from two rings at once.
            # No gpsimd SWDGE: Q7 descriptor rings add ~1us/transfer setup
            # and a multi-us teardown wait in the TileContext epilogue.
            plan = [
                (0, 1024, nc.scalar),
                (1024, 1024, nc.sync),
                (2048, 2048, nc.scalar),
                (4096, 2048, nc.sync),
                (6144, 2048, nc.sync),
            ]
            c = 8192
            while c < NCOLS:
                plan.append((c, SLAB_N, nc.sync))
                c += SLAB_N

            t_act = 0.0  # greedy engine balance
            t_dve = 0.0
            t = 0
            for c0, ncols_slab, eng in plan:
                xb = io_pool.tile([P, ncols_slab], dt.float8e4, tag="xb")
                eng.dma_start(
                    out=xb[:, 0:ncols_slab],
                    in_=x_dram[:, c0 : c0 + ncols_slab],
                )

                for i in range(ncols_slab // TILE_N):
                    pt = psum_pool.tile([P, TILE_N], dt.float32, tag="pt")
                    # PSUM accumulation targets cannot span banks (ISA check):
                    # each 512-col bank is filled by 2 tile-matmuls, one per
                    # 64-partition PSUM half d, fed from SBUF half
                    # s = (bank + d) % 2 so consecutive banks rotate through
                    # all 4 PE tiles (max tile-level parallelism).
                    for half in range(2):
                        gb = t * 2 + half  # global bank index
                        c0 = i * TILE_N + half * BANK_N
                        for d in range(2):
                            s = (gb + d) % 2
                            nc.tensor.matmul(
                                out=pt[64 * d : 64 * d + 64, half * BANK_N : (half + 1) * BANK_N],
                                lhsT=wt[64 * s : 64 * s + 64, :],
                                rhs=xb[64 * s : 64 * s + 64, c0 : c0 + BANK_N],
                                start=True,
                                stop=True,
                            )
                    if t_act + ACT_NS <= t_dve + DVE_NS:
                        t_act += ACT_NS
                        dump = dump_pool.tile([P, TILE_N], dt.bfloat16, tag="da")
                        nc.scalar.activation(
                            out=dump[:],
                            in_=pt[:],
                            func=mybir.ActivationFunctionType.Abs,
                            accum_out=acc[:, t : t + 1],
                        )
                    else:
                        t_dve += DVE_NS
                        nc.vector.tensor_reduce(
                            out=acc[:, t : t + 1],
                            in_=pt[:],
                            axis=mybir.AxisListType.X,
                            op=mybir.AluOpType.add,
                            apply_absolute_value=True,
                        )
                    t += 1
            assert t == TILES

            accsum = acc_pool.tile([P, 1], dt.float32)
            nc.vector.tensor_reduce(
                out=accsum[:],
                in_=acc[:],
                axis=mybir.AxisListType.X,
                op=mybir.AluOpType.add,
            )
            nc.sync.dma_start(out=out_dram[:, :], in_=accsum[:])

    nc.compile()
    return nc


_NC_CACHE: bass.Bass | None = None


def _get_program() -> bass.Bass:
    global _NC_CACHE
    if _NC_CACHE is None:
        _NC_CACHE = _build_program()
    return _NC_CACHE


def _prep_core(a8: np.ndarray, c: int) -> np.ndarray:
    """[N_BLOCKS*C, 8, 8] fp8 -> this core's [128, NCOLS] moving layout."""
    nb = BLOCKS_PER_CORE
    v = a8[c * nb : (c + 1) * nb].reshape(G, BPG, 8, 8)
    return np.ascontiguousarray(v.transpose(0, 2, 1, 3).reshape(P, NCOLS))


def _build_weights() -> np.ndarray:
    """[128, 64] stationary: each 64-partition half holds blockdiag8(H)."""
    H = _hadamard8()
    W64 = np.zeros((64, 64), dtype=np.float32)
    for a in range(8):
        W64[a * 8 : (a + 1) * 8, a * 8 : (a + 1) * 8] = H
    return np.tile(W64, (2, 1)).astype(F8)


def run(original: np.ndarray, pred: np.ndarray, trace: bool = False, **kwargs):
    """Shard, run on 8 cores, return (scalar result, BassKernelResults)."""
    d8 = (
        np.asarray(original, dtype=np.float32) - np.asarray(pred, dtype=np.float32)
    ).reshape(-1, 8, 8).astype(F8)
    w = _build_weights()
    in_maps = []
    for c in range(N_CORES):
        in_maps.append({"x": _prep_core(d8, c), "w": w})
    nc = _get_program()
    res = run_bass_kernel_spmd(
        nc, in_maps, core_ids=list(range(N_CORES)), trace=trace, **kwargs
    )
    total = np.float64(0.0)
    for r in res.results:
        total += r["out"].astype(np.float64).sum()
    return np.array(total, dtype=np.float32), res


def kernel(original: np.ndarray, pred: np.ndarray) -> np.ndarray:
    out, _ = run(original, pred, trace=False)
    return out


# revision 4
# speedup vs baseline: 1.1095x; 1.1095x over previous
"""SATD-style custom loss on 8 Trainium2 NeuronCores.

Computes sum(|H8 @ (original - pred)|) where H8 is the 8x8 Sylvester
Hadamard matrix applied along dim -2 of [B, C, 8, 8] blocks.

Strategy (v5): pure data parallel over the block-batch dim (8 shards).
  - Host computes the residual d = original - pred and casts it to
    fp8e4 (e4m3): 8x less HBM traffic than shipping both fp32 tensors
    (the residual is the only thing the loss depends on; quantization
    costs ~3e-4 rel err on the loss, gate is 2e-2).
  - Layout per core: [128, 98304] fp8, partition k = g*8+j (16
    block-groups x 8 Hadamard input rows), col n = b*8+w.
  - DMA: SP HWDGE + gpsimd SWDGE alternate 512KB slab transfers.
  - PE: 64x64 array packing (4 independent PE tiles). The H8 transform
    is block-diagonal with K=8, so a 64x64 tile covers 8 block-groups.
    Tile (s,d) reads SBUF partitions 64s:64s+64 and writes PSUM
    partitions 64d:64d+64; per 512-col PSUM bank, 2 tile-matmuls
    (one per PSUM half) fill the bank from the two SBUF halves.
    4 tiles run concurrently -> 2x PE column throughput, so the PE
    does not starve the drain engines even when HAM-throttled.
    (32x32 with 16 tiles would be 4x, but SBUF base partition 96 -
    PE quadrant 3 - is broken on TRN2, so 32-partition stripe s=3
    cannot feed the PE.)
  - PSUM drain: [128, 1024] fp32 chunks (two adjacent banks, a single
    contiguous PSUM read) abs-summed by ACT (activation(Abs, accum_out))
    and DVE (tensor_reduce add with apply_absolute_value), greedily
    balanced by measured per-chunk cost. TRN2 allows only one PSUM
    input per instruction and gpsimd cannot touch PSUM, so these two
    engines are the drain ceiling.
  - Final [128,1] partials DMA'd out; host sums 8x128 in float64.
"""

import numpy as np
import ml_dtypes

import concourse.bacc as bacc
import concourse.bass as bass
import concourse.mybir as mybir
from concourse.bass_utils import run_bass_kernel_spmd
from concourse.tile import TileContext

# Problem shape (hardcoded; kernel.py must be self-contained).
N_BLOCKS = 524288
C = 3
N_CORES = 8
BLOCKS_PER_CORE = (N_BLOCKS // N_CORES) * C  # 196608 8x8 blocks
G = 16  # block-groups stacked on partitions (16 * 8 = 128)
BPG = BLOCKS_PER_CORE // G  # 12288 blocks per group
P = 128
NCOLS = BPG * 8  # 98304 moving columns (b*8+w)
BANK_N = 512  # fp32 cols per PSUM bank
TILE_N = 1024  # drain granularity: two adjacent banks
TILES = NCOLS // TILE_N  # 96 double-bank tiles
SLAB_N = 4096  # cols per full DMA slab (512 KiB)

ACT_NS = 1278.0  # measured: activation + accumulator-read per [128,1024]
DVE_NS = 1181.0  # measured tensor_reduce on [128,1024] from PSUM

F8 = ml_dtypes.float8_e4m3  # matches mybir dt.float8e4


def _hadamard8() -> np.ndarray:
    H = np.array([[1.0]], dtype=np.float32)
    while H.shape[0] < 8:
        H = np.block([[H, H], [H, -H]])
    return H


def _build_program() -> bass.Bass:
    nc = bacc.Bacc("TRN2", debug=False, num_devices=N_CORES)
    dt = mybir.dt

    x_dram = nc.declare_dram_parameter("x", [P, NCOLS], dt.float8e4, isOutput=False)
    w_dram = nc.declare_dram_parameter("w", [P, 64], dt.float8e4, isOutput=False)
    out_dram = nc.declare_dram_parameter("out", [P, 1], dt.float32, isOutput=True)

    with TileContext(nc) as tc:
        with (
            tc.tile_pool(name="io", bufs=8) as io_pool,
            tc.tile_pool(name="wpool", bufs=1) as w_pool,
            tc.tile_pool(name="dump", bufs=4) as dump_pool,
            tc.tile_pool(name="acc", bufs=1) as acc_pool,
            tc.tile_pool(name="psum", bufs=4, space="PSUM") as psum_pool,
        ):
            wt = w_pool.tile([P, 64], dt.float8e4)
            nc.sync.dma_start(out=wt[:], in_=w_dram[:, :])

            acc = acc_pool.tile([P, TILES], dt.float32)

            # slab plan: first slabs arrive in small pieces so the first
            # matmuls + drains start as soon as possible; rest are 4096-col
            # (512 KiB) transfers alternating between the two DMA paths
            # (SP HWDGE, gpsimd SWDGE). ACT/DVE never trigger DMA - they
            # are drain-bound.
            # All x DMA on HWDGE rings: sync carries the stream; scalar
            # (idle until its first drain chunk ~4us in) carries the first
            # two pieces so the pipeline primes from two rings at once.
            # No gpsimd SWDGE: Q7 descriptor rings add ~1us/transfer setup
            # and a multi-us teardown wait in the TileContext epilogue.
            plan = [
                (0, 1024, nc.scalar),
                (1024, 1024, nc.sync),
                (2048, 2048, nc.scalar),
                (4096, 2048, nc.sync),
                (6144, 2048, nc.sync),
            ]
            c = 8192
            while c < NCOLS:
                plan.append((c, SLAB_N, nc.sync))
                c += SLAB_N

            t_act = 0.0  # greedy engine balance
            t_dve = 0.0
            t = 0
            for c0, ncols_slab, eng in plan:
                xb = io_pool.tile([P, ncols_slab], dt.float8e4, tag="xb")
                eng.dma_start(
                    out=xb[:, 0:ncols_slab],
                    in_=x_dram[:, c0 : c0 + ncols_slab],
                )

                for i in range(ncols_slab // TILE_N):
                    pt = psum_pool.tile([P, TILE_N], dt.float32, tag="pt")
                    # PSUM accumulation targets cannot span banks (ISA check):
                    # each 512-col bank is filled by 2 tile-matmuls, one per
                    # 64-partition PSUM half d, fed from SBUF half
                    # s = (bank + d) % 2 so consecutive banks rotate through
                    # all 4 PE tiles (max tile-level parallelism).
                    for half in range(2):
                        gb = t * 2 + half  # global bank index
                        c0 = i * TILE_N + half * BANK_N
                        for d in range(2):
                            s = (gb + d) % 2
                            nc.tensor.matmul(
                                out=pt[64 * d : 64 * d + 64, half * BANK_N : (half + 1) * BANK_N],
                                lhsT=wt[64 * s : 64 * s + 64, :],
                                rhs=xb[64 * s : 64 * s + 64, c0 : c0 + BANK_N],
                                start=True,
                                stop=True,
                            )
                    if t_act + ACT_NS <= t_dve + DVE_NS:
                        t_act += ACT_NS
                        dump = dump_pool.tile([P, TILE_N], dt.bfloat16, tag="da")
                        nc.scalar.activation(
                            out=dump[:],
                            in_=pt[:],
                            func=mybir.ActivationFunctionType.Abs,
                            accum_out=acc[:, t : t + 1],
                        )
                    else:
                        t_dve += DVE_NS
                        nc.vector.tensor_reduce(
                            out=acc[:, t : t + 1],
                            in_=pt[:],
                            axis=mybir.AxisListType.X,
                            op=mybir.AluOpType.add,
                            apply_absolute_value=True,
                        )
                    t += 1
            assert t == TILES

            accsum = acc_pool.tile([P, 1], dt.float32)
            nc.vector.tensor_reduce(
                out=accsum[:],
                in_=acc[:],
                axis=mybir.AxisListType.X,
                op=mybir.AluOpType.add,
            )
            nc.sync.dma_start(out=out_dram[:, :], in_=accsum[:])

    nc.compile()
    return nc


_NC_CACHE: bass.Bass | None = None


def _get_program() -> bass.Bass:
    global _NC_CACHE
    if _NC_CACHE is None:
        _NC_CACHE = _build_program()
    return _NC_CACHE


def _prep_core(a8: np.ndarray, c: int) -> np.ndarray:
    """[N_BLOCKS*C, 8, 8] fp8 -> this core's [128, NCOLS] moving layout."""
    nb = BLOCKS_PER_CORE
    v = a8[c * nb : (c + 1) * nb].reshape(G, BPG, 8, 8)
    return np.ascontiguousarray(v.transpose(0, 2, 1, 3).reshape(P, NCOLS))


def _build_weights() -> np.ndarray:
    """[128, 64] stationary: each 64-partition half holds blockdiag8(H)."""
    H = _hadamard8()
    W64 = np.zeros((64, 64), dtype=np.float32)
    for a in range(8):
        W64[a * 8 : (a + 1) * 8, a * 8 : (a + 1) * 8] = H
    return np.tile(W64, (2, 1)).astype(F8)


def run(original: np.ndarray, pred: np.ndarray, trace: bool = False, **kwargs):
    """Shard, run on 8 cores, return (scalar result, BassKernelResults)."""
    d8 = (
        np.asarray(original, dtype=np.float32) - np.asarray(pred, dtype=np.float32)
    ).reshape(-1, 8, 8).astype(F8)
    w = _build_weights()
    in_maps = []
    for c in range(N_CORES):
        in_maps.append({"x": _prep_core(d8, c), "w": w})
    nc = _get_program()
    res = run_bass_kernel_spmd(
        nc, in_maps, core_ids=list(range(N_CORES)), trace=trace, **kwargs
    )
    total = np.float64(0.0)
    for r in res.results:
        total += r["out"].astype(np.float64).sum()
    return np.array(total, dtype=np.float32), res


def kernel(original: np.ndarray, pred: np.ndarray) -> np.ndarray:
    out, _ = run(original, pred, trace=False)
    return out
